# revision 9
# baseline (speedup 1.0000x reference)
"""Single-query attention pooling kernel for Trainium2 (Bass/Tile).

Problem: hidden [32, 4096, 768] f32, querys [1, 768] f32
  scores = einsum("bsh,qh->bs", hidden, querys)
  attn   = softmax(scores, axis=-1)
  out    = einsum("bs,bsh->bh", attn, hidden)          # [32, 768]

Strategy (8 NeuronCores, SPMD, no collectives):
  - Shard batch dim: 4 batches per core; querys replicated.
  - Per batch (4096 x 768 f32 = 12.6 MB) the data fits in SBUF, so we do a
    single HBM pass: DMA 32 chunk tiles [128, 768], compute scores for each
    chunk with one fused DVE tensor_tensor_reduce (mult + row-reduce) against
    a partition-broadcast copy of q, then softmax over the resident scores
    [128, 32], then 2x32 accumulating PE matvecs (lhsT = exp weights column,
    rhs = resident chunk tile) into PSUM [1, 384] halves, normalize, DMA out.
  - Chunk tiles from a shared pool (many bufs) so batch b+1's DMA overlaps
    batch b's softmax + matvec phase.
"""

import numpy as np

import concourse.bass as bass
import concourse.mybir as mybir
import concourse.tile as tile
from concourse.bass_utils import run_bass_kernel_spmd
from concourse.masks import make_identity

B, S, H = 32, 4096, 768
N_CORES = 8
B_PER = B // N_CORES            # 4 batches per core
P = 128                         # partitions
N_CHUNKS = S // P               # 32 sequence chunks per batch
H_HALF = H // 2                 # 384 (fits one PSUM bank in f32)
CHUNK_BUFS = 48                 # resident chunk slots (48 * 3KB = 144KB/part)

# dtype used for the weighted-sum matvec streaming through the PE.
# float32  : exact, 4 cycles/row
# float32r : single-pass fp32, 1 cycle/row at N>=256 (lower internal precision)
MATVEC_DT = mybir.dt.float32


def _body(ctx, tc: tile.TileContext, out: bass.AP, hidden: bass.AP,
          querys: bass.AP):
    nc = tc.nc
    f32 = mybir.dt.float32
    Alu = mybir.AluOpType
    Act = mybir.ActivationFunctionType

    chunks = ctx.enter_context(tc.tile_pool(name="chunks", bufs=CHUNK_BUFS))
    scratch = ctx.enter_context(tc.tile_pool(name="scratch", bufs=2))
    singles = ctx.enter_context(tc.tile_pool(name="singles", bufs=1))
    stats = ctx.enter_context(tc.tile_pool(name="stats", bufs=4))
    outs = ctx.enter_context(tc.tile_pool(name="outs", bufs=2))
    psum_r = ctx.enter_context(tc.tile_pool(name="psum_r", bufs=4, space="PSUM"))
    psum_s = ctx.enter_context(tc.tile_pool(name="psum_s", bufs=1, space="PSUM"))

    # q broadcast to all 128 partitions (one small DMA, reused all kernel)
    q_rep = singles.tile([P, H], f32, tag="q_rep")
    nc.sync.dma_start(out=q_rep, in_=querys.to_broadcast([P, H]))
    ones_col = singles.tile([P, 1], f32, tag="ones_col")
    nc.vector.memset(ones_col, 1.0)
    ones_row = singles.tile([1, P], f32, tag="ones_row")
    nc.vector.memset(ones_row, 1.0)
    identity = singles.tile([P, P], f32, tag="ident")
    make_identity(nc, identity)

    for b in range(B_PER):
        # ---- load batch + scores ----
        # scores[:, c] = sum_h tile_c[:, h] * q[h]: DVE does the elementwise
        # multiply; ScalarE's activation-accumulate does the free-dim reduce.
        scores = stats.tile([P, N_CHUNKS], f32, tag="scores")
        tiles = []
        for c in range(N_CHUNKS):
            t = chunks.tile([P, H], f32, tag="chunk")
            nc.sync.dma_start(out=t, in_=hidden[b, c * P:(c + 1) * P, :])
            tiles.append(t)
            tmp = scratch.tile([P, H], f32, tag="tmp")
            nc.vector.tensor_mul(out=tmp, in0=t, in1=q_rep)
            tmp2 = scratch.tile([P, H], f32, tag="tmp2")
            nc.scalar.activation(out=tmp2, in_=tmp, func=Act.Copy,
                                 accum_out=scores[:, c:c + 1])

        # ---- softmax over the 4096 resident scores ----
        # global max: free-dim reduce, PE transpose [128,1]->[1,128],
        # free-dim reduce (negated), broadcast back via K=1 matmul
        rowmax = stats.tile([P, 1], f32, tag="rowmax")
        nc.vector.reduce_max(out=rowmax, in_=scores, axis=mybir.AxisListType.X)
        rmT = psum_s.tile([1, P], f32, tag="rmT")
        nc.tensor.transpose(rmT, rowmax, identity)
        negm = stats.tile([1, 1], f32, tag="negm")
        nc.vector.tensor_reduce(out=negm, in_=rmT, axis=mybir.AxisListType.X,
                                op=Alu.max, negate=True)
        negm_b = psum_s.tile([P, 1], f32, tag="negm_b")
        nc.tensor.matmul(negm_b, lhsT=ones_row, rhs=negm, start=True, stop=True)
        negm_sb = stats.tile([P, 1], f32, tag="negm_sb")
        nc.scalar.copy(out=negm_sb, in_=negm_b)
        w = stats.tile([P, N_CHUNKS], f32, tag="w")
        nc.scalar.activation(out=w, in_=scores, func=Act.Exp,
                             bias=negm_sb, scale=1.0)

        # denominator: column sums via PE, then row-reduce
        pl = psum_s.tile([1, N_CHUNKS], f32, tag="pl")
        nc.tensor.matmul(pl, lhsT=ones_col, rhs=w, start=True, stop=True)
        lsum = stats.tile([1, 1], f32, tag="lsum")
        nc.vector.reduce_sum(out=lsum, in_=pl, axis=mybir.AxisListType.X)
        rl = stats.tile([1, 1], f32, tag="rl")
        nc.vector.reciprocal(out=rl, in_=lsum)

        # ---- weighted sum: out[1, H] = sum_c w[:, c]^T @ tile_c ----
        pr0 = psum_r.tile([1, H_HALF], f32, tag="pr")
        pr1 = psum_r.tile([1, H_HALF], f32, tag="pr")
        if MATVEC_DT == f32:
            w_mv, t_mv = w, tiles
        else:
            w_mv = w.bitcast(MATVEC_DT)
            t_mv = [t.bitcast(MATVEC_DT) for t in tiles]
        for c in range(N_CHUNKS):
            first, last = c == 0, c == N_CHUNKS - 1
            nc.tensor.matmul(pr0, lhsT=w_mv[:, c:c + 1],
                             rhs=t_mv[c][:, 0:H_HALF], start=first, stop=last)
            nc.tensor.matmul(pr1, lhsT=w_mv[:, c:c + 1],
                             rhs=t_mv[c][:, H_HALF:H], start=first, stop=last)

        # ---- normalize + store ----
        res = outs.tile([1, H], f32, tag="res")
        nc.vector.tensor_scalar_mul(out=res[:, 0:H_HALF], in0=pr0, scalar1=rl)
        nc.vector.tensor_scalar_mul(out=res[:, H_HALF:H], in0=pr1, scalar1=rl)
        nc.sync.dma_start(out=out[b:b + 1, :], in_=res)


def build_bass() -> bass.Bass:
    nc = bass.Bass("TRN2", target_bir_lowering=False, debug=False,
                   enable_asserts=False, num_devices=N_CORES)
    hidden = nc.dram_tensor("hidden", (B_PER, S, H), mybir.dt.float32,
                            kind="ExternalInput").ap()
    querys = nc.dram_tensor("querys", (1, H), mybir.dt.float32,
                            kind="ExternalInput").ap()
    out = nc.dram_tensor("out", (B_PER, H), mybir.dt.float32,
                         kind="ExternalOutput").ap()
    with tile.TileContext(nc) as tc:
        from contextlib import ExitStack
        with ExitStack() as ctx:
            _body(ctx, tc, out, hidden, querys)
    split_multi_waits(nc)
    return nc


def split_multi_waits(nc: bass.Bass, max_keep: int = 1) -> int:
    """Walrus in this container encodes at most one sync-wait command on most
    ISA instructions ("Too many sync wait commands" otherwise). Hoist extra
    waits onto standalone InstEventSemaphore instructions inserted just
    before the owning instruction on the same engine — semantics preserved,
    since the engine executes its stream in order."""
    n_split = 0
    for f in nc.m.functions:
        for blk in f.blocks:
            new_insts = []
            for inst in blk.instructions:
                si = inst.sync_info
                waits = list(si.on_wait) if (si is not None and si.on_wait) else []
                if len(waits) > max_keep:
                    for w in waits[:-max_keep]:
                        ev = mybir.InstEventSemaphore(
                            name=f"I-{nc.next_id()}-waitsplit", ins=[], outs=[])
                        ev.engine = inst.engine
                        ev.sync_info = mybir.SyncInfo(on_wait=[w], on_update=[])
                        nc.register_instruction(ev, overwrite=True)
                        new_insts.append(ev)
                        n_split += 1
                    si.on_wait = waits[-max_keep:]
                new_insts.append(inst)
            blk.instructions[:] = new_insts
    return n_split


_NC = None


def _get_nc() -> bass.Bass:
    global _NC
    if _NC is None:
        _NC = build_bass()
    return _NC


def run(hidden: np.ndarray, querys: np.ndarray, **spmd_kwargs):
    """Run on 8 cores; returns (full_output [32, 768], BassKernelResults)."""
    hidden = np.ascontiguousarray(np.asarray(hidden, dtype=np.float32))
    querys = np.ascontiguousarray(np.asarray(querys, dtype=np.float32))
    assert hidden.shape == (B, S, H) and querys.shape == (1, H)
    in_maps = [
        {"hidden": np.ascontiguousarray(hidden[i * B_PER:(i + 1) * B_PER]),
         "querys": querys}
        for i in range(N_CORES)
    ]
    r = run_bass_kernel_spmd(_get_nc(), in_maps,
                             core_ids=list(range(N_CORES)), **spmd_kwargs)
    out = np.concatenate([m["out"] for m in r.results], axis=0)
    return np.ascontiguousarray(out, dtype=np.float32), r


def kernel(hidden: np.ndarray, querys: np.ndarray) -> np.ndarray:
    out, _ = run(hidden, querys)
    return out
